# revision 98
# baseline (speedup 1.0000x reference)
"""Trainium2 Bass kernel for nn_BasicDeconvolutionBlock (two-phase design).

Reference computation:
    gathered = feats[in_map]                         # [K, M, Cin]
    contrib  = einsum('kmc,kcd->kmd', gathered, W)   # [K, M, Cout]
    out      = zeros([n_out, Cout]).at[out_map].add(contrib)
    y        = relu(batchnorm(out))                  # batch stats over n_out rows

Strategy (8 NeuronCores, SPMD, output-row sharding):
  Host routes each (k, m) pair to the core owning its output row
  (row blocks of n_out/8, ~169k pairs/core), orders the pairs by
  (feats-chunk, k, out-tile) with per-(chunk,k,out-tile) "cells" padded to
  even length, groups (chunk,k) padded to 128.

  Phase A (gather-GEMM): SWDGE dma_gather (transpose) pulls feats rows
  (fp16, 256B) channel-major; per-128-slot matmul against W[k] (fp16);
  PSUM -> fp16 slab (scalar engine Copy) -> contiguous HBM contrib table
  (slot-major 128B rows, in window tensors of 64k slots so phase B's
  int16 gather indices stay in range).  No scatter-add, no occurrence
  rounds.

  Phase B (gather-reduce): contrib rows are fetched in out-tile order as
  PAIRS (256B descriptors = 2 rows, halving descriptor count; cells are
  even-aligned so pairs never straddle cells).  One-hot S matrices
  ([128 pairs x 128 rows], fp16) are built on-chip with a single
  broadcast is_equal against an iota tile per gather call; matmul
  lhsT=S, rhs=gathered pair block accumulates the segmented scatter-add
  directly in PSUM per 128-row out-tile.  Tiles accumulate across the
  window passes into an SBUF fp32 slab.

  BN: per-tile ones-matmul row sums + sum of squares, [2,64] AllReduce
  across 8 cores, batched normalize + ReLU, output shard [25088,64] fp32.
"""

import numpy as np

BN_EPS = 1e-5
SEG = 896            # max descriptors per SWDGE call (Q7 ucode ring limit)
CHUNK = 32768        # int16 gather index range per feats chunk
WSLOTS = 65536       # contrib-table window: 32768 pairs of slots


def _lazy():
    global F32, F16, I16, mybir, bacc, tile
    import sys
    for p in ("/opt/trn_rl_repo",):
        if p not in sys.path:
            sys.path.insert(0, p)
    from concourse import bacc as _bacc, mybir as _mybir
    import concourse.tile as _tile
    mybir, bacc, tile = _mybir, _bacc, _tile
    F32 = mybir.dt.float32
    F16 = mybir.dt.float16
    I16 = mybir.dt.int16


def _roundup(x, m):
    return (x + m - 1) // m * m


def _plan(in_map, out_map, n_out, n_cores, rows_per_core, chunk, wslots):
    """Host-side routing. Returns a dict plan + per-core packed arrays."""
    K, M = in_map.shape
    tiles = _roundup(rows_per_core, 128) // 128
    wpairs = wslots // 2
    in_flat = np.asarray(in_map).ravel().astype(np.int64)
    out_flat = np.asarray(out_map).ravel().astype(np.int64)
    k_idx = np.repeat(np.arange(K, dtype=np.int64), M)
    core = out_flat // rows_per_core
    row_local = out_flat - core * rows_per_core
    t_idx = row_local >> 7
    r128 = row_local & 127
    chnk = in_flat // chunk
    idx_local = in_flat - chnk * chunk
    nchunk = int(chnk.max()) + 1
    NG = nchunk * K
    NCELLS = NG * tiles
    cell = (chnk * K + k_idx) * tiles + t_idx

    sizes = np.zeros((n_cores, NCELLS), np.int64)
    for c in range(n_cores):
        sizes[c] = np.bincount(cell[core == c], minlength=NCELLS)
    ce = (sizes + 1) // 2 * 2                       # cell sizes even-padded
    gsz = ce.reshape(n_cores, NG, tiles).sum(-1)
    gcap = _roundup(gsz.max(0), 128)                # [NG] shared
    gcap = np.where((gcap // 128) % 2 == 1, gcap + 128, gcap)  # even tiles
    gstart = np.zeros(NG + 1, np.int64)
    gstart[1:] = np.cumsum(gcap)
    AS = int(gstart[-1])                            # total A slots

    # ---- A-table layout units ----
    # vertical 8-tile units: table_pos = ubase + p*8 + t (1KB write runs);
    # horizontal remainder units (<8 tiles): table_pos = ubase + t*128 + p.
    k_of_group = np.tile(np.arange(K, dtype=np.int64), nchunk)
    units = []   # (ubase, nt, vertical, k, chunk_id)
    for g in range(NG):
        tiles_g = int(gcap[g]) // 128
        ub = int(gstart[g])
        for u in range(tiles_g // 8):
            units.append([ub + u * 1024, 8, True,
                          int(k_of_group[g]), g // K])
        rem = tiles_g % 8
        if rem:
            units.append([ub + (tiles_g // 8) * 1024, rem, rem % 2 == 0,
                          int(k_of_group[g]), g // K])
    ubounds = np.array([u[0] for u in units] + [AS], np.int64)

    # windows cut at unit boundaries, each <= wslots slots
    wb = [0]
    while wb[-1] < AS:
        i = int(np.searchsorted(ubounds, wb[-1] + wslots, side="right")) - 1
        nxt = int(ubounds[i]) if int(ubounds[i]) > wb[-1] else AS
        nxt = min(nxt, AS)
        assert nxt > wb[-1]
        wb.append(nxt)
    nwin = len(wb) - 1
    wb = np.array(wb, np.int64)
    wbp = wb // 2                                    # pair-space bounds
    for u in units:
        u.append(int(np.searchsorted(wb, u[0], side="right")) - 1)

    ce3 = ce.reshape(n_cores, NG, tiles)
    cstart = (np.cumsum(ce3, axis=2) - ce3
              + gstart[None, :NG, None])            # [cores, NG, tiles]
    cstart_f = cstart.reshape(n_cores, NCELLS)

    order = np.lexsort((cell, core))
    cell_s, core_s = cell[order], core[order]
    key = core_s * NCELLS + cell_s
    n = len(key)
    first = np.ones(n, bool)
    first[1:] = key[1:] != key[:-1]
    gs = np.maximum.accumulate(np.where(first, np.arange(n), 0))
    pos = np.arange(n) - gs
    Apos_s = cstart_f[core_s, cell_s] + pos

    gidxA = np.zeros((n_cores, AS), np.int16)
    rowsA = np.full((n_cores, AS), 255, np.int16)
    gidxA[core_s, Apos_s] = idx_local[order].astype(np.int16)
    rowsA[core_s, Apos_s] = r128[order].astype(np.int16)

    # ---- B stream ----
    p0 = cstart_f // 2
    cnt2 = ce.reshape(n_cores, NCELLS) // 2          # pairs per cell
    t_of_cell = np.tile(np.arange(tiles, dtype=np.int64), NG)

    sizes_B = np.zeros((n_cores, nwin, tiles), np.int64)
    for c in range(n_cores):
        for w in range(nwin):
            lo, hi = int(wbp[w]), int(wbp[w + 1])
            ov = np.clip(np.minimum(p0[c] + cnt2[c], hi)
                         - np.maximum(p0[c], lo), 0, None)
            sizes_B[c, w] = ov.reshape(NG, tiles).sum(0)
    CB = sizes_B.max(0)                              # [nwin, tiles] shared
    sec = CB.sum(1)
    secpad = _roundup(sec, 128)
    wstart = np.zeros(nwin + 1, np.int64)
    wstart[1:] = np.cumsum(secpad)
    BS = int(wstart[-1])                             # total B pairs
    P_wt = np.cumsum(CB, axis=1) - CB + wstart[:nwin, None]

    bidx = np.zeros((n_cores, BS), np.int16)
    brow = np.full((n_cores, BS, 2), 255, np.int16)
    for c in range(n_cores):
        tot = int(cnt2[c].sum())
        if tot == 0:
            continue
        cums = np.cumsum(cnt2[c]) - cnt2[c]
        ap_all = (np.repeat(p0[c], cnt2[c])
                  + np.arange(tot) - np.repeat(cums, cnt2[c]))
        t_all = np.repeat(t_of_cell, cnt2[c])
        w_all = np.searchsorted(wbp, ap_all, side="right") - 1
        key2 = w_all * tiles + t_all
        o2 = np.lexsort((ap_all, key2))
        k2, a2, t2, w2 = key2[o2], ap_all[o2], t_all[o2], w_all[o2]
        f2 = np.ones(tot, bool)
        f2[1:] = k2[1:] != k2[:-1]
        gs2 = np.maximum.accumulate(np.where(f2, np.arange(tot), 0))
        pos2 = np.arange(tot) - gs2
        bpos = P_wt[w2, t2] + pos2
        bidx[c, bpos] = (a2 - wbp[w2]).astype(np.int16)
        brow[c, bpos, 0] = rowsA[c, a2 * 2]
        brow[c, bpos, 1] = rowsA[c, a2 * 2 + 1]

    # entries (block, w, t) + per-(w,t) entry spans
    entries = []
    ent_span = {}
    for w in range(nwin):
        for t in range(tiles):
            if CB[w, t] == 0:
                continue
            b0 = int(P_wt[w, t]) // 128
            b1 = (int(P_wt[w, t]) + int(CB[w, t]) + 127) // 128
            ent_span[(w, t)] = (len(entries), len(entries) + b1 - b0)
            entries.extend((b, w, t) for b in range(b0, b1))
    NENT = len(entries)

    metaI = np.full((n_cores, NENT, 128, 2), 255, np.int16)
    for e, (b, w, t) in enumerate(entries):
        lo = max(b * 128, int(P_wt[w, t]))
        hi = min((b + 1) * 128, int(P_wt[w, t]) + int(CB[w, t]))
        if hi > lo:
            metaI[:, e, lo - b * 128:hi - b * 128, :] = brow[:, lo:hi, :]
    meta = metaI.transpose(0, 2, 1, 3).astype(np.float16)

    # ---- segmentation ----
    # A calls: cut at chunk-section and window boundaries, then SEG slots.
    csec = [int(gstart[ci * K]) for ci in range(nchunk + 1)]
    bounds = sorted(set(csec + wb.tolist()))
    uslot = np.array([u[0] for u in units], np.int64)
    a_calls = []   # (slot0, ns, chunk_id, win, [(k, unit_id, t_local)])
    for lo, hi in zip(bounds[:-1], bounds[1:]):
        s = lo
        while s < hi:
            ns = min(SEG, hi - s)
            tinfo = []
            for j in range(ns // 128):
                sj = s + 128 * j
                uid = int(np.searchsorted(uslot, sj, side="right")) - 1
                tinfo.append((units[uid][3], uid,
                              (sj - units[uid][0]) // 128))
            a_calls.append(
                (s, ns, int(np.searchsorted(csec, s, side="right") - 1),
                 int(np.searchsorted(wb, s, side="right")) - 1, tinfo))
            s += ns

    # B calls: per window section, SEG-pair chunks (128-multiples)
    b_calls = []   # (pair0, np_, w)
    for w in range(nwin):
        s = int(wstart[w])
        hi = int(wstart[w + 1])
        while s < hi:
            np_ = min(SEG, hi - s)
            b_calls.append((s, np_, w))
            s += np_
    call_lo = np.array([c[0] for c in b_calls])
    call_ent = [[] for _ in b_calls]
    for e, (b, w, t) in enumerate(entries):
        ci = int(np.searchsorted(call_lo, b * 128, side="right") - 1)
        assert b_calls[ci][0] <= b * 128 < b_calls[ci][0] + b_calls[ci][1]
        call_ent[ci].append(e)
    ne_max = max((len(x) for x in call_ent), default=0)

    first_w = {}
    last_w = {}
    for t in range(tiles):
        for w in range(nwin):
            if CB[w, t] > 0:
                if t not in first_w:
                    first_w[t] = w
                last_w[t] = w

    table_of_stream = np.empty(AS, np.int64)
    for (ub, nt, vert, _k, _ch, _w) in units:
        n_ = nt * 128
        if vert:
            q = (np.arange(128)[None, :] * nt
                 + np.arange(nt)[:, None])           # [t, p] -> table off
            table_of_stream[ub:ub + n_] = ub + q.reshape(-1)
        else:
            table_of_stream[ub:ub + n_] = ub + np.arange(n_)

    gidxA_w = np.zeros((n_cores, 128, AS // 16), np.int16)
    gidxB_w = np.zeros((n_cores, 128, BS // 16), np.int16)
    for c in range(n_cores):
        gidxA_w[c] = np.tile(
            gidxA[c][table_of_stream].reshape(-1, 16).T, (8, 1))
        gidxB_w[c] = np.tile(bidx[c].reshape(-1, 16).T, (8, 1))

    plan = dict(
        K=K, tiles=tiles, nchunk=nchunk, nwin=nwin, AS=AS, BS=BS,
        chunk=chunk, wslots=wslots, rows_per_core=rows_per_core,
        a_calls=a_calls, b_calls=b_calls, entries=entries,
        ent_span=ent_span, call_ent=call_ent, ne_max=ne_max,
        first_w=first_w, last_w=last_w, NENT=NENT, n_out=int(n_out),
        units=units, wb=wb.tolist(), table_of_stream=table_of_stream,
    )
    arrays = dict(gidxA=gidxA_w, gidxB=gidxB_w, meta=meta)
    return plan, arrays


def _build(plan, n_cores, ftab_rows):
    """Trace the Bass program."""
    _lazy()
    nc = bacc.Bacc("TRN2", target_bir_lowering=False, debug=False)

    K, tiles, nwin = plan["K"], plan["tiles"], plan["nwin"]
    AS, BS, NENT = plan["AS"], plan["BS"], plan["NENT"]
    ne_max = max(plan["ne_max"], 1)
    wslots = plan["wslots"]
    chunk = plan["chunk"]
    n_out = plan["n_out"]
    Cout = 64
    rows_pad = tiles * 128

    ftab = nc.dram_tensor("ftab", [ftab_rows, 128], F16, kind="ExternalInput")
    wt = nc.dram_tensor("wt", [128, K * Cout], F16, kind="ExternalInput")
    gidxA = nc.dram_tensor("gidxA", [128, AS // 16], I16, kind="ExternalInput")
    gidxB = nc.dram_tensor("gidxB", [128, BS // 16], I16, kind="ExternalInput")
    meta = nc.dram_tensor("meta", [128, NENT, 2], F16, kind="ExternalInput")
    iota2 = nc.dram_tensor("iota2", [128, 128, 2], F16,
                           kind="ExternalInput")
    gb = nc.dram_tensor("gb", [2, Cout], F32, kind="ExternalInput")
    atabs = [nc.dram_tensor(f"atab{w}", [wslots // 2, 128], F16)
             for w in range(nwin)]
    cc_in = nc.dram_tensor("cc_in", [2, Cout], F32)
    cc_out = nc.dram_tensor("cc_out", [2 * n_cores, Cout], F32,
                            addr_space="Shared")
    cc_mask = nc.dram_tensor("cc_mask", [2 * n_cores, 2], F32,
                             kind="ExternalInput")
    y = nc.dram_tensor("y", [rows_pad, Cout], F16, kind="ExternalOutput")

    units = plan["units"]
    wb = plan["wb"]
    entries = plan["entries"]
    ent_span = plan["ent_span"]
    call_ent = plan["call_ent"]
    first_w = plan["first_w"]
    last_w = plan["last_w"]
    # full 4-tile groups: every (w, t) cell must be populated
    assert tiles % 4 == 0
    for w in range(nwin):
        for t in range(tiles):
            assert (w, t) in ent_span, (w, t)
    assert all(v == 0 for v in first_w.values())
    assert all(v == nwin - 1 for v in last_w.values())
    tg_stat_first = 0
    tg_stat_last = tiles // 4 - 1
    ent_first = {}
    ent_last = {}
    for (w, t), (e0, e1) in ent_span.items():
        ent_first[(w, t)] = e0
        ent_last[(w, t)] = e1 - 1

    with tile.TileContext(nc) as tc:
        with (
            tc.tile_pool(name="const", bufs=1) as cpool,
            tc.tile_pool(name="agix", bufs=4) as agix,
            tc.tile_pool(name="ag", bufs=6) as agp,
            tc.tile_pool(name="aslab", bufs=10) as aslab,
            tc.tile_pool(name="bgix", bufs=4) as bgix,
            tc.tile_pool(name="bg", bufs=8) as bgp,
            tc.tile_pool(name="bmeta", bufs=4) as bmeta,
            tc.tile_pool(name="bs", bufs=3) as bspool,
            tc.tile_pool(name="slab", bufs=1) as slabpool,
        ):
            w_sb = cpool.tile([128, K * Cout], F16, tag="w")
            nc.sync.dma_start(out=w_sb[:, :], in_=wt[:, :])
            iota_sb = cpool.tile([128, 128, 2], F16, tag="iota")
            nc.sync.dma_start(out=iota_sb[:, :, :], in_=iota2[:, :, :])
            out_slab = slabpool.tile([128, tiles, Cout], F16, tag="slab")

            a_by_w = [[] for _ in range(nwin)]
            for call in plan["a_calls"]:
                a_by_w[call[3]].append(call)
            b_by_w = [[] for _ in range(nwin)]
            for ci, call in enumerate(plan["b_calls"]):
                b_by_w[call[2]].append((ci, call))

            psum_of = {}
            psum_of_u = {}
            pending = []
            apsum, bpsum = [], []

            ABATCH = 17

            def a_batches(w):
                calls = a_by_w[w]
                outs = []
                for i0 in range(0, len(calls), ABATCH):
                    outs.append(_mk_a(calls[i0:i0 + ABATCH], w, i0))
                return outs

            def _mk_a(batch, w, i0):
                def go():
                    bs0 = batch[0][0]
                    bs1 = batch[-1][0] + batch[-1][1]
                    gib = agix.tile([128, ABATCH * SEG // 16], I16, tag="agi")
                    ldq = nc.scalar if (i0 // ABATCH) % 2 else nc.sync
                    ldq.dma_start(
                        out=gib[:, :(bs1 - bs0) // 16],
                        in_=gidxA[:, bs0 // 16:bs1 // 16])
                    for (s0, ns, ch, _w, tinfo) in batch:
                        g = agp.tile([128, 1, SEG], F16, tag="ag")
                        nc.gpsimd.dma_gather(
                            out_ap=g[:, :, :ns],
                            in_ap=ftab[ch * chunk:(ch + 1) * chunk, :],
                            idxs_ap=gib[:, (s0 - bs0) // 16:
                                        (s0 - bs0 + ns) // 16],
                            num_idxs=ns,
                            num_idxs_reg=ns,
                            elem_size=128,
                            transpose=True,
                        )
                        for j, (k, uid, tl) in enumerate(tinfo):
                            if uid not in psum_of_u:
                                psum_of_u[uid] = apsum[0].tile(
                                    [128, 8, Cout], F32, tag="aps",
                                    name=f"aps_{uid}")
                            psu = psum_of_u[uid]
                            nc.tensor.matmul(
                                out=psu[:, tl, :],
                                lhsT=g[:, 0, j * 128:(j + 1) * 128],
                                rhs=w_sb[:, k * Cout:(k + 1) * Cout],
                                start=True, stop=True,
                            )
                            ub, ntu, vert, _k2, _c2, uw = units[uid]
                            if tl == ntu - 1:
                                sl = aslab.tile(
                                    [128, 8, Cout], F16, tag="asl",
                                    name=f"asl_{uid}")
                                nc.scalar.activation(
                                    out=sl[:, :ntu, :], in_=psu[:, :ntu, :],
                                    func=mybir.ActivationFunctionType.Copy)
                                be = (ub - wb[uw]) * 64
                                ne_ = ntu * 128 * 64
                                flat = atabs[uw][:, :].flatten()
                                if vert:
                                    oap = flat[be:be + ne_].rearrange(
                                        "(p t c) -> p t c", t=ntu, c=64)
                                else:
                                    oap = flat[be:be + ne_].rearrange(
                                        "(t p c) -> p t c", p=128, c=64)
                                wq = (nc.sync if (s0 // SEG) % 2
                                      else nc.scalar)
                                wq.dma_start(out=oap, in_=sl[:, :ntu, :])
                                del psum_of_u[uid]
                return go

            BBATCH = 13

            def b_batches(w):
                calls = b_by_w[w]
                outs = []
                for i0 in range(0, len(calls), BBATCH):
                    outs.append(_mk_b(calls[i0:i0 + BBATCH], w, i0))
                return outs

            def _mk_b(bat, w, i0):
                def go():
                    bp0 = bat[0][1][0]
                    bp1 = bat[-1][1][0] + bat[-1][1][1]
                    gib = bgix.tile([128, BBATCH * SEG // 16], I16, tag="bgi")
                    ldq = nc.scalar if (i0 // BBATCH) % 2 else nc.sync
                    ldq.dma_start(
                        out=gib[:, :(bp1 - bp0) // 16],
                        in_=gidxB[:, bp0 // 16:bp1 // 16])
                    be_lo = call_ent[bat[0][0]][0]
                    be_hi = call_ent[bat[-1][0]][-1] + 1
                    mtb = bmeta.tile([128, BBATCH * ne_max, 2], F16, tag="bm")
                    nc.scalar.dma_start(
                        out=mtb[:, :be_hi - be_lo, :],
                        in_=meta[:, be_lo:be_hi, :])
                    for (ci, (pair0, np_, _w)) in bat:
                        nb = np_ // 128
                        g = bgp.tile([128, SEG // 128, 128], F16, tag="bg")
                        nc.gpsimd.dma_gather(
                            out_ap=g[:, :nb, :],
                            in_ap=atabs[w][:, :],
                            idxs_ap=gib[:, (pair0 - bp0) // 16:
                                        (pair0 - bp0 + np_) // 16],
                            num_idxs=np_,
                            num_idxs_reg=np_,
                            elem_size=128,
                        )
                        ents = call_ent[ci]
                        if not ents:
                            continue
                        ne = len(ents)
                        e_lo = ents[0]
                        st = bspool.tile(
                            [128, 128, ne_max, 2], F16, tag="bsl")
                        nc.vector.tensor_tensor(
                            out=st[:, :, :ne, :],
                            in0=mtb[:, e_lo - be_lo:e_lo - be_lo + ne, :]
                            .unsqueeze(1).broadcast_to([128, 128, ne, 2]),
                            in1=iota_sb[:, :, :].unsqueeze(2)
                            .broadcast_to([128, 128, ne, 2]),
                            op=mybir.AluOpType.is_equal,
                        )
                        for e in ents:
                            b, we, t = entries[e]
                            tg = t // 4
                            if (we, tg) not in psum_of:
                                psum_of[(we, tg)] = bpsum[0].tile(
                                    [128, 4, Cout], F32, tag="bps",
                                    name=f"bps_{we}_{tg}")
                            ps = psum_of[(we, tg)]
                            last = e == ent_last[(we, t)]
                            for eo in range(2):
                                nc.tensor.matmul(
                                    out=ps[:, t % 4, :],
                                    lhsT=st[:, :, e - e_lo, eo],
                                    rhs=g[:, b - pair0 // 128,
                                          eo * 64:(eo + 1) * 64],
                                    start=(e == ent_first[(we, t)]
                                           and eo == 0),
                                    stop=(last and eo == 1),
                                )
                            if last and t == tg * 4 + 3:
                                pending.append((we, tg, ps))
                                del psum_of[(we, tg)]
                    for (we, tg, ps) in pending:
                        sl_ = out_slab[:, tg * 4:tg * 4 + 4, :]
                        if we == 0:
                            nc.scalar.activation(
                                out=sl_, in_=ps[:, :, :],
                                func=mybir.ActivationFunctionType.Copy)
                        else:
                            nc.vector.tensor_tensor(
                                out=sl_, in0=sl_, in1=ps[:, :, :],
                                op=mybir.AluOpType.add)
                        if we == nwin - 1:
                            sqt = aslab.tile(
                                [128, 4, Cout], F16, tag="sq",
                                name=f"sq_{tg}")
                            nc.scalar.activation(
                                out=sqt[:, :, :], in_=sl_,
                                func=mybir.ActivationFunctionType.Square)
                            nc.tensor.matmul(
                                out=stat_ps[:, 0:256],
                                lhsT=ones1[:, :],
                                rhs=sl_,
                                start=(tg == tg_stat_first),
                                stop=(tg == tg_stat_last))
                            nc.tensor.matmul(
                                out=stat_ps[:, 256:512],
                                lhsT=ones1[:, :],
                                rhs=sqt[:, :, :],
                                start=(tg == tg_stat_first),
                                stop=(tg == tg_stat_last))
                    pending.clear()
                return go

            with (
                tc.tile_pool(name="apsum", bufs=2, space="PSUM") as apsum_,
                tc.tile_pool(name="bpsum", bufs=5, space="PSUM") as bpsum_,
                tc.tile_pool(name="spsum", bufs=1, space="PSUM") as spsum_,
            ):
                apsum.append(apsum_)
                bpsum.append(bpsum_)
                stat_ps = spsum_.tile([1, 512], F32, tag="stat")
                ones1 = cpool.tile([128, 1], F16, tag="ones1")
                nc.vector.memset(ones1[:, :], 1.0)
                for go in a_batches(0):
                    go()
                for w in range(1, nwin):
                    A, B = a_batches(w), b_batches(w - 1)
                    na, nb = len(A), len(B)
                    ia = ib = 0
                    while ia < na or ib < nb:
                        if ia < na and (ib >= nb or ia * nb <= ib * na):
                            A[ia]()
                            ia += 1
                        else:
                            B[ib]()
                            ib += 1
                for go in b_batches(nwin - 1):
                    go()
                stf = cpool.tile([1, 512], F32, tag="stf")
                nc.vector.tensor_copy(out=stf[:, :], in_=stat_ps[:, :])
                # fold 4 tile-columns into one (sum region and sq region)
                st0 = cpool.tile([1, Cout], F32, tag="st0")
                st1 = cpool.tile([1, Cout], F32, tag="st1")
                nc.vector.tensor_copy(out=st0[:, :], in_=stf[:, 0:64])
                for jj in range(1, 4):
                    nc.vector.tensor_tensor(
                        out=st0[:, :], in0=st0[:, :],
                        in1=stf[:, jj * 64:(jj + 1) * 64],
                        op=mybir.AluOpType.add)
                nc.vector.tensor_copy(out=st1[:, :], in_=stf[:, 256:320])
                for jj in range(1, 4):
                    nc.vector.tensor_tensor(
                        out=st1[:, :], in0=st1[:, :],
                        in1=stf[:, 256 + jj * 64:256 + (jj + 1) * 64],
                        op=mybir.AluOpType.add)
                nc.sync.dma_start(out=cc_in[0:1, :], in_=st0[:, :])
                nc.sync.dma_start(out=cc_in[1:2, :], in_=st1[:, :])
                nc.gpsimd.collective_compute(
                    "AllGather",
                    mybir.AluOpType.bypass,
                    ins=[cc_in[:, :]],
                    outs=[cc_out[:, :]],
                    replica_groups=[list(range(n_cores))],
                )

            # ---- BN + ReLU ----
            with (
                tc.tile_pool(name="bn", bufs=4) as bnp,
                tc.tile_pool(name="bnps", bufs=2, space="PSUM") as bnps,
            ):
                cc_sb = bnp.tile([2 * n_cores, Cout], F32, tag="ccsb")
                nc.sync.dma_start(out=cc_sb[:, :], in_=cc_out[:, :])
                mask_sb = bnp.tile([2 * n_cores, 2], F32, tag="ccm")
                nc.scalar.dma_start(out=mask_sb[:, :], in_=cc_mask[:, :])
                red_ps = bnps.tile([1, 2 * Cout], F32, tag="red")
                nc.tensor.matmul(
                    out=red_ps[:, 0:Cout], lhsT=mask_sb[:, 0:1],
                    rhs=cc_sb[:, :], start=True, stop=True)
                nc.tensor.matmul(
                    out=red_ps[:, Cout:2 * Cout], lhsT=mask_sb[:, 1:2],
                    rhs=cc_sb[:, :], start=True, stop=True)
                red_sb = bnp.tile([1, 2 * Cout], F32, tag="redsb")
                nc.vector.tensor_copy(out=red_sb[:, :], in_=red_ps[:, :])
                gs0 = red_sb[:, 0:Cout]
                gs1 = red_sb[:, Cout:2 * Cout]
                gam_t = bnp.tile([1, Cout], F32, tag="gam")
                bet_t = bnp.tile([1, Cout], F32, tag="bet")
                nc.sync.dma_start(out=gam_t[:, :], in_=gb[0:1, :])
                nc.sync.dma_start(out=bet_t[:, :], in_=gb[1:2, :])

                inv_n = 1.0 / float(n_out)
                mean_t = bnp.tile([1, Cout], F32, tag="mean")
                ex2_t = bnp.tile([1, Cout], F32, tag="ex2")
                var_t = bnp.tile([1, Cout], F32, tag="var")
                sd_t = bnp.tile([1, Cout], F32, tag="sd")
                rs_t = bnp.tile([1, Cout], F32, tag="rs")
                a_t = bnp.tile([1, Cout], F32, tag="a")
                b_t = bnp.tile([1, Cout], F32, tag="b")
                nc.vector.tensor_scalar_mul(mean_t[:, :], gs0, inv_n)
                nc.vector.tensor_scalar_mul(ex2_t[:, :], gs1, inv_n)
                nc.vector.tensor_tensor(
                    out=var_t[:, :], in0=mean_t[:, :], in1=mean_t[:, :],
                    op=mybir.AluOpType.mult)
                nc.vector.tensor_tensor(
                    out=var_t[:, :], in0=ex2_t[:, :], in1=var_t[:, :],
                    op=mybir.AluOpType.subtract)
                nc.vector.tensor_scalar_add(var_t[:, :], var_t[:, :], BN_EPS)
                nc.scalar.activation(
                    out=sd_t[:, :], in_=var_t[:, :],
                    func=mybir.ActivationFunctionType.Sqrt)
                nc.vector.reciprocal(out=rs_t[:, :], in_=sd_t[:, :])
                nc.vector.tensor_tensor(
                    out=a_t[:, :], in0=gam_t[:, :], in1=rs_t[:, :],
                    op=mybir.AluOpType.mult)
                nc.vector.tensor_tensor(
                    out=b_t[:, :], in0=mean_t[:, :], in1=a_t[:, :],
                    op=mybir.AluOpType.mult)
                nc.vector.tensor_tensor(
                    out=b_t[:, :], in0=bet_t[:, :], in1=b_t[:, :],
                    op=mybir.AluOpType.subtract)
                ones_row = bnp.tile([1, 128], F32, tag="ones_row")
                nc.vector.memset(ones_row[:, :], 1.0)
                a_full = bnp.tile([128, Cout], F16, tag="afull")
                b_full = bnp.tile([128, Cout], F16, tag="bfull")
                ab_ps = bnps.tile([128, Cout], F32, tag="abps")
                nc.tensor.matmul(
                    out=ab_ps[:, :], lhsT=ones_row[:, :], rhs=a_t[:, :],
                    start=True, stop=True)
                nc.vector.tensor_copy(out=a_full[:, :], in_=ab_ps[:, :])
                nc.tensor.matmul(
                    out=ab_ps[:, :], lhsT=ones_row[:, :], rhs=b_t[:, :],
                    start=True, stop=True)
                nc.vector.tensor_copy(out=b_full[:, :], in_=ab_ps[:, :])
                # batched normalize + relu + write, pipelined in chunks
                yv = y[:, :].flatten().rearrange(
                    "(t p c) -> p t c", p=128, c=64)
                nchk = 14
                step = tiles // nchk
                for ci_ in range(nchk):
                    tlo = ci_ * step
                    thi = tiles if ci_ == nchk - 1 else (ci_ + 1) * step
                    nn = thi - tlo
                    sl_ = out_slab[:, tlo:thi, :]
                    nc.vector.tensor_tensor(
                        out=sl_, in0=sl_,
                        in1=a_full[:, :].unsqueeze(1)
                        .broadcast_to([128, nn, Cout]),
                        op=mybir.AluOpType.mult)
                    nc.vector.tensor_tensor(
                        out=sl_, in0=sl_,
                        in1=b_full[:, :].unsqueeze(1)
                        .broadcast_to([128, nn, Cout]),
                        op=mybir.AluOpType.add)
                    y16 = bnp.tile([128, step, Cout], F16, tag="y16")
                    nc.scalar.activation(
                        out=y16[:, :nn, :], in_=sl_,
                        func=mybir.ActivationFunctionType.Relu)
                    wq = nc.sync if ci_ % 2 else nc.scalar
                    wq.dma_start(out=yv[:, tlo:thi, :], in_=y16[:, :nn, :])

    nc.compile()
    return nc


def _prepare(feats, W, gamma, beta, in_map, out_map, n_out,
             n_cores=8, dup_safe=False, expand=1):
    """Host prep shared by kernel() and tests. Returns (nc, in_maps, plan)."""
    _lazy()
    n_out = int(n_out)
    K, Cin, Cout = W.shape
    assert Cin == 64 and Cout == 64
    rows_per_core = n_out // n_cores
    assert rows_per_core * n_cores == n_out

    in_map = np.asarray(in_map, dtype=np.int64)
    out_map = np.asarray(out_map, dtype=np.int64)
    feats = np.asarray(feats, dtype=np.float32)
    W = np.asarray(W, dtype=np.float32)

    plan, arrays = _plan(in_map, out_map, n_out, n_cores, rows_per_core,
                         CHUNK, WSLOTS)

    ftab_rows = _roundup(feats.shape[0], CHUNK)
    ftab = np.zeros((ftab_rows, 128), dtype=np.float16)
    ftab[:feats.shape[0], :64] = feats.astype(np.float16)
    wt = np.zeros((128, K * 64), dtype=np.float16)
    wt[:64, :] = W.transpose(1, 0, 2).reshape(64, K * 64).astype(np.float16)
    gb = np.stack([np.asarray(gamma, np.float32),
                   np.asarray(beta, np.float32)])
    iota2 = np.broadcast_to(np.arange(128, dtype=np.float32)[None, :, None],
                            (128, 128, 2)).astype(np.float16)
    cc_mask = np.zeros((2 * n_cores, 2), np.float32)
    cc_mask[0::2, 0] = 1.0
    cc_mask[1::2, 1] = 1.0

    nc = _build(plan, n_cores, ftab_rows)
    in_maps = [
        dict(ftab=ftab, wt=wt, gb=gb, iota2=iota2, cc_mask=cc_mask,
             gidxA=arrays["gidxA"][c], gidxB=arrays["gidxB"][c],
             meta=arrays["meta"][c])
        for c in range(n_cores)
    ]
    return nc, in_maps, plan


def kernel(feats, W, gamma, beta, in_map, out_map, n_out):
    _lazy()
    from concourse.bass_utils import run_bass_kernel_spmd

    n_cores = 8
    nc, in_maps, plan = _prepare(
        feats, W, gamma, beta, in_map, out_map, n_out, n_cores)
    res = run_bass_kernel_spmd(nc, in_maps, list(range(n_cores)))
    rows = plan["rows_per_core"]
    out = np.concatenate(
        [res.results[c]["y"][:rows] for c in range(n_cores)], axis=0)
    return out.astype(np.float32)
